# revision 43
# baseline (speedup 1.0000x reference)
"""Additive (Bahdanau) attention on Trainium2, data-parallel over batch on 8 NeuronCores.

Math (per batch b):
    qp = queries @ W_q                     [Tq, H]
    kp = keys @ W_k                        [Tk, H]
    scores[q,k] = sum_h v[h] * tanh(qp[q,h] + kp[k,h] + b[h])
    masked softmax over k (k < seq_len[b]), then out = align @ keys.

Design (per core, 4 batch "slots" with compile-time key-lengths L_slots,
ordered [3rd-longest, longest, 2nd-longest, shortest] so the pipeline fills
fast and drains on the smallest slot):
  - ALL host-side layout prep in bf16: transposed keys (ktall), natural
    mask-zeroed keys||maskones chunks (knall), and a const tensor cx holding
    W_q||b rows, [W_k|W_k], v2 and the pair-duplicated transposed queries
    (qTx-even/odd with a trailing ones row) - zero device transposes.
  - qp2+b [128, 128] per slot-pair in TWO K=65 matmuls (even-query half to
    psum[0:64], odd to psum[64:128]; the 65th contraction row is ones x b,
    folding the bias) + one DVE cast to bf16. kp per slot: one matmul
    (lhsT=[W_k|W_k] fills both 64-partition halves) + one DVE cast.
  - S[h2, j*L+k] = kpb2 + qp2[:, 2j+par] via DVE tensor_tensor adds in bf16
    (2x mode), tanh on ACT (the roofline: 32*sum(L) lane-cycles @1.2GHz).
  - scores^T[k, 2j+t] via PE: lhsT = S_tanh[:, j*L+off:+w] stationary,
    rhs = v2 [128, 2] -> psum[w, 64*ci+2j:+2]; all k-chunks of a slot land
    in one PSUM tile, so ONE exp per slot (no bias: masking lives in
    knall's zeroed rows), bf16 out feeding the final matmuls directly.
  - out_un[q, h]|rowsum: PSUM-accumulated E-chunks @ knall chunks; DVE
    reciprocal + scale, bf16 DMA out (host casts back to f32).
  - tanh j-splits are sized so each slot's LAST split is small: the exp
    (which needs every j) then trails the last tanh by only a few score
    matmuls, and the ACT queue interleaves the next slot's tanh to cover
    even that.

Batches are sorted by seq_len and dealt so each core gets one batch per slot
rank; slot k-length = max over the 8 batches of that rank (rounded to 2).
All cores run the identical program on different data (SPMD).
"""

import sys

_REPO = "/opt/trn_rl_repo"
if _REPO not in sys.path:
    sys.path.insert(0, _REPO)

import numpy as np

try:
    from ml_dtypes import bfloat16 as np_bf16
except ImportError:  # pragma: no cover
    import jax.numpy as _jnp

    np_bf16 = _jnp.bfloat16

from concourse import bacc, tile
import concourse.mybir as mybir
from concourse import bass_utils

B, TQ, TK, H = 32, 64, 256, 64
NCORES = 8
SLOTS = 4
F32 = mybir.dt.float32
BF16 = mybir.dt.bfloat16
TANH = mybir.ActivationFunctionType.Tanh
EXP = mybir.ActivationFunctionType.Exp
KXC = 66  # keys | maskones | pad, per 128-row chunk of knall
QPREF = 256  # qTx for slot-pair g=1, prefixed to knall

_prog_cache: dict = {}


def _roundup(x, m):
    return ((x + m - 1) // m) * m


def _nch(L):
    return (L + 127) // 128


# per-slot tanh/adds j-splits: last split small so exp trails it closely
_SLOT_SPLITS = [
    [(0, 8), (8, 20), (20, 32)],
    [(0, 12), (12, 24), (24, 32)],
    [(0, 12), (12, 26), (26, 32)],
    [(0, 16), (16, 26), (26, 32)],
]


def _build(L_slots):
    nc = bacc.Bacc(
        "TRN2",
        target_bir_lowering=False,
        debug=False,
        enable_asserts=False,
        num_devices=NCORES,
    )
    nchs = [_nch(L) for L in L_slots]
    KT = sum(L_slots)
    KN = QPREF + sum(nchs) * KXC
    CX = 194 + 256  # wqb65 64 | wk2 128 | v2 2 | qTx-even/odd g0

    cx_d = nc.dram_tensor("cx", [128, CX], BF16, kind="ExternalInput").ap()
    kt_d = nc.dram_tensor("ktall", [H, KT], BF16, kind="ExternalInput").ap()
    kn_d = nc.dram_tensor("knall", [128, KN], BF16, kind="ExternalInput").ap()
    o_d = nc.dram_tensor("out", [SLOTS, TQ, H], BF16, kind="ExternalOutput").ap()

    ktoff = [sum(L_slots[:s]) for s in range(SLOTS)]
    knoff = [QPREF + sum(nchs[:s]) * KXC for s in range(SLOTS)]
    Lmax = max(L_slots)
    nchmax = max(nchs)

    with tile.TileContext(nc) as tc:
        with (
            tc.tile_pool(name="const", bufs=1) as cpool,
            tc.tile_pool(name="qpool", bufs=2) as qpool,
            tc.tile_pool(name="kpb", bufs=2) as kpool,
            tc.tile_pool(name="spool", bufs=2) as spool,
            tc.tile_pool(name="epool", bufs=2) as epool,
            tc.tile_pool(name="opool", bufs=2) as opool,
            tc.tile_pool(name="qpp", bufs=2, space="PSUM") as qpp,
            tc.tile_pool(name="kpp", bufs=2, space="PSUM") as kpp,
            tc.tile_pool(name="scp", bufs=2, space="PSUM") as scp,
            tc.tile_pool(name="oup", bufs=2, space="PSUM") as oup,
        ):
            # ---- prefetch in three DMAs on three different engines so the
            # three hardware queues stream in parallel (hottest via ACT,
            # whose BSP startup finishes first)
            # 4-byte queue-warm DMA: pays the HWDGE first-use init before
            # the real cx transfer rings its doorbell
            dwarm = cpool.tile([1, 2], BF16, name="dwarm", tag="dwarm")
            nc.scalar.dma_start(out=dwarm, in_=cx_d[0:1, 0:2])
            cx_sb = cpool.tile([128, CX], BF16, name="cx_sb", tag="cx")
            nc.scalar.dma_start(out=cx_sb, in_=cx_d)
            kt_sb = cpool.tile([H, KT], BF16, name="kt_sb", tag="kt")
            nc.sync.dma_start(out=kt_sb, in_=kt_d)
            kn_sb = cpool.tile([128, KN], BF16, name="kn_sb", tag="kn")
            nc.gpsimd.dma_start(out=kn_sb, in_=kn_d)

            # tiny activation so the ACT table set loads during the DMA wait
            scr = cpool.tile([1, 2], F32, name="scr", tag="scr")
            nc.vector.memset(scr, 0.0)
            nc.scalar.activation(scr, scr, TANH)

            wqb65 = cx_sb[0:65, 0:64]
            wk2_sb = cx_sb[0:H, 64:192]
            v2_sb = cx_sb[:, 192:194]
            qtx = {
                (0, 0): cx_sb[0:65, 194:322],
                (0, 1): cx_sb[0:65, 322:450],
                (1, 0): kn_sb[0:65, 0:128],
                (1, 1): kn_sb[0:65, 128:256],
            }

            qp2_sb = [None, None]

            def qproj_mm(g):
                ps = qpp.tile([128, 128], F32, name=f"qpps{g}", tag="qp")
                nc.tensor.matmul(ps[0:64, :], lhsT=wqb65, rhs=qtx[(g, 0)])
                nc.tensor.matmul(ps[64:128, :], lhsT=wqb65, rhs=qtx[(g, 1)])
                return ps

            def qproj_dve(g, ps):
                qp2 = qpool.tile([128, 128], BF16, name=f"qp2_{g}", tag="qp2")
                nc.vector.tensor_copy(qp2, ps)
                qp2_sb[g] = qp2

            kpb2s = {}

            def kproj_mm(s):
                L = L_slots[s]
                ps = kpp.tile([128, Lmax], F32, name=f"kpps{s}", tag="kp")
                nc.tensor.matmul(
                    ps[:, 0:L], lhsT=wk2_sb, rhs=kt_sb[:, ktoff[s] : ktoff[s] + L]
                )
                return ps

            def kproj_dve(s, ps):
                L = L_slots[s]
                kpb2 = kpool.tile([128, Lmax], BF16, name=f"kpb2_{s}", tag="kpb2")
                nc.vector.tensor_copy(kpb2[:, 0:L], ps[:, 0:L])
                kpb2s[s] = kpb2

            # ---- per-slot stages
            S_alls, S_tanhs, sc_pss, E_sbs, out_pss = {}, {}, {}, {}, {}

            def adds(s, j0, j1):
                L = L_slots[s]
                if s not in S_alls:
                    S_alls[s] = spool.tile(
                        [128, 32 * Lmax], BF16, name=f"S{s}", tag="S"
                    )
                S_all = S_alls[s]
                kpb2 = kpb2s[s]
                qp2 = qp2_sb[s // 2]
                qoff = 64 * (s % 2)
                nj = j1 - j0
                in0 = (
                    kpb2[:, 0:L]
                    .rearrange("c (k two) -> c k two", two=2)
                    .unsqueeze(1)
                    .broadcast_to([128, nj, L // 2, 2])
                )
                in1 = (
                    qp2[:, qoff + 2 * j0 : qoff + 2 * j1]
                    .rearrange("c (j two) -> c j two", two=2)
                    .unsqueeze(2)
                    .broadcast_to([128, nj, L // 2, 2])
                )
                s_out = S_all[:, j0 * L : j1 * L].rearrange(
                    "c (j k two) -> c j k two", two=2, k=L // 2
                )
                nc.vector.tensor_add(s_out, in0, in1)

            def tanh(s, j0, j1):
                L = L_slots[s]
                if s not in S_tanhs:
                    S_tanhs[s] = spool.tile(
                        [128, 32 * Lmax], BF16, name=f"T{s}", tag="T"
                    )
                nc.scalar.activation(
                    S_tanhs[s][:, j0 * L : j1 * L],
                    S_alls[s][:, j0 * L : j1 * L],
                    TANH,
                )

            def scores(s, j0, j1):
                L = L_slots[s]
                nch = nchs[s]
                S_tanh = S_tanhs[s]
                if s not in sc_pss:
                    sc_pss[s] = scp.tile(
                        [128, 64 * nchmax], F32, name=f"sc{s}", tag="sc"
                    )
                    wl = L - 128 * (nch - 1)
                    if nch > 1 and wl < 128:
                        # exp reads the whole tile; pre-zero the last chunk's
                        # columns (its matmuls overwrite rows [0:wl])
                        nc.vector.memset(
                            sc_pss[s][:, 64 * (nch - 1) : 64 * nch], 0.0
                        )
                sc_ps = sc_pss[s]
                for ci in range(nch):
                    off = 128 * ci
                    w = min(128, L - off)
                    for j in range(j0, j1):
                        nc.tensor.matmul(
                            sc_ps[0:w, 64 * ci + 2 * j : 64 * ci + 2 * j + 2],
                            lhsT=S_tanh[:, j * L + off : j * L + off + w],
                            rhs=v2_sb,
                            start=True,
                            stop=True,
                        )

            def expo(s, j0=0, j1=32):
                # exp of query-columns [2*j0, 2*j1) across all k-chunks; a
                # partial call lets the last slot's exp trail its last tanh
                # by only a few score matmuls
                L = L_slots[s]
                nch = nchs[s]
                wmax = min(128, L)
                if s not in E_sbs:
                    E_sbs[s] = epool.tile(
                        [128, 64 * nchmax], BF16, name=f"E{s}", tag="E"
                    )
                E = E_sbs[s]
                if (j0, j1) == (0, 32):
                    nc.scalar.activation(
                        E[0:wmax, 0 : 64 * nch], sc_pss[s][0:wmax, 0 : 64 * nch], EXP
                    )
                    return
                for ci in range(nch):
                    c0, c1 = 64 * ci + 2 * j0, 64 * ci + 2 * j1
                    nc.scalar.activation(
                        E[0:wmax, c0:c1], sc_pss[s][0:wmax, c0:c1], EXP
                    )

            def outmm(s):
                L = L_slots[s]
                nch = nchs[s]
                E = E_sbs[s]
                out_ps = oup.tile([TQ, H + 1], F32, name=f"ops{s}", tag="ou")
                out_pss[s] = out_ps
                for ci in range(nch):
                    w = min(128, L - 128 * ci)
                    kno = knoff[s] + ci * KXC
                    nc.tensor.matmul(
                        out_ps,
                        lhsT=E[0:w, 64 * ci : 64 * ci + 64],
                        rhs=kn_sb[0:w, kno : kno + H + 1],
                        start=(ci == 0),
                        stop=(ci == nch - 1),
                    )

            def norm(s):
                out_ps = out_pss[s]
                recip = opool.tile([TQ, 1], F32, name=f"rc{s}", tag="rc")
                nc.vector.reciprocal(recip, out_ps[:, H : H + 1])
                out_sb = opool.tile([TQ, H], BF16, name=f"osb{s}", tag="osb")
                nc.vector.tensor_scalar_mul(out_sb, out_ps[:, 0:H], recip)
                nc.sync.dma_start(out=o_d[s], in_=out_sb)

            sp = _SLOT_SPLITS

            # ---- hand-interleaved schedule.
            # ACT: W,T0abc,T1a,E0,T1b,T1c,T2a,E1,T2b,T3a,T2c,T3b,E2,T3c,E3
            # DVE: casts just-in-time, adds in ACT order, norms at the end.
            # PE:  projections early, scores in j-split order, outmms asap.
            kps0 = kproj_mm(0)
            kproj_dve(0, kps0)
            qps0 = qproj_mm(0)
            qproj_dve(0, qps0)
            kps1 = kproj_mm(1)

            for j0, j1 in sp[0]:
                adds(0, j0, j1)
                tanh(0, j0, j1)
                scores(0, j0, j1)
            kproj_dve(1, kps1)
            qps1 = qproj_mm(1)
            kps2 = kproj_mm(2)

            adds(1, *sp[1][0])
            tanh(1, *sp[1][0])
            expo(0)
            scores(1, *sp[1][0])
            adds(1, *sp[1][1])
            tanh(1, *sp[1][1])
            scores(1, *sp[1][1])
            qproj_dve(1, qps1)
            adds(1, *sp[1][2])
            tanh(1, *sp[1][2])
            scores(1, *sp[1][2])
            kproj_dve(2, kps2)
            kps3 = kproj_mm(3)
            outmm(0)

            adds(2, *sp[2][0])
            tanh(2, *sp[2][0])
            expo(1)
            scores(2, *sp[2][0])
            adds(2, *sp[2][1])
            tanh(2, *sp[2][1])
            scores(2, *sp[2][1])
            kproj_dve(3, kps3)
            outmm(1)

            adds(3, *sp[3][0])
            tanh(3, *sp[3][0])
            adds(2, *sp[2][2])
            tanh(2, *sp[2][2])
            scores(2, *sp[2][2])
            adds(3, *sp[3][1])
            tanh(3, *sp[3][1])
            scores(3, *sp[3][0])
            scores(3, *sp[3][1])
            expo(2)
            adds(3, *sp[3][2])
            tanh(3, *sp[3][2])
            scores(3, *sp[3][2])
            outmm(2)
            expo(3)
            outmm(3)
            for s in range(SLOTS):
                norm(s)

    nc.compile()
    return nc


def _get_prog(L_slots):
    if L_slots not in _prog_cache:
        _prog_cache[L_slots] = _build(L_slots)
    return _prog_cache[L_slots]


def _plan(seq_len_flat):
    sl = np.asarray(seq_len_flat).reshape(-1).astype(np.int64)
    order = np.argsort(-sl, kind="stable")
    grp = [order[NCORES * r : NCORES * (r + 1)] for r in range(SLOTS)]
    slot_of_rank = [1, 2, 0, 3]  # rank r (0=longest) -> slot index
    assign = np.zeros((NCORES, SLOTS), dtype=np.int64)
    L_slots = [0] * SLOTS
    for r in range(SLOTS):
        s = slot_of_rank[r]
        assign[:, s] = grp[r]
        L = int(max(1, sl[grp[r]].max()))
        L_slots[s] = min(TK, _roundup(L, 2))
    return tuple(L_slots), assign, sl


def _make_in_maps(queries, keys, sl, assign, W_q, W_k, v, b, L_slots):
    W_q = np.asarray(W_q, np.float32)
    W_k = np.asarray(W_k, np.float32)
    vv = np.asarray(v, np.float32).reshape(-1)
    bb = np.asarray(b, np.float32).reshape(-1)
    nchs = [_nch(L) for L in L_slots]
    KT = sum(L_slots)
    KN = QPREF + sum(nchs) * KXC
    CX = 194 + 256

    base = np.zeros((128, CX), np.float32)
    base[0:H, 0:H] = W_q
    base[H, 0:H] = bb
    base[0:H, 64 : 64 + H] = W_k
    base[0:H, 64 + H : 192] = W_k
    base[0:H, 192] = vv
    base[H:128, 193] = vv

    def qtx_cols(q):
        # [65, 64]: row h = queries[2*(m//2) (+1 for odd), h].T; row 64 = 1
        up = np.zeros((65, 64), np.float32)
        lo = np.zeros((65, 64), np.float32)
        up[0:H] = np.repeat(q[0::2], 2, axis=0).T
        lo[0:H] = np.repeat(q[1::2], 2, axis=0).T
        up[H] = 1.0
        lo[H] = 1.0
        return up, lo

    in_maps = []
    for c in range(NCORES):
        cx = base.copy()
        knall = np.zeros((128, KN), np.float32)
        for g in range(2):
            for half in range(2):
                s = 2 * g + half
                up, lo = qtx_cols(queries[assign[c, s]])
                if g == 0:
                    cx[0:65, 194 + 64 * half : 258 + 64 * half] = up
                    cx[0:65, 322 + 64 * half : 386 + 64 * half] = lo
                else:
                    knall[0:65, 64 * half : 64 * half + 64] = up
                    knall[0:65, 128 + 64 * half : 192 + 64 * half] = lo
        ktall = np.zeros((H, KT), np.float32)
        kto, kno = 0, QPREF
        for s, L in enumerate(L_slots):
            b_i = assign[c, s]
            kk = keys[b_i]  # [256, 64]
            lv = int(min(sl[b_i], L))
            ktall[:, kto : kto + lv] = kk[0:lv].T
            for ci in range(nchs[s]):
                off = 128 * ci
                w = min(128, L - off)
                vw = max(0, min(lv - off, w))
                if vw > 0:
                    knall[0:vw, kno : kno + H] = kk[off : off + vw]
                    knall[0:vw, kno + H] = 1.0
                kno += KXC
            kto += L
        in_maps.append(
            {
                "cx": cx.astype(np_bf16),
                "ktall": ktall.astype(np_bf16),
                "knall": knall.astype(np_bf16),
            }
        )
    return in_maps


def _run_spmd(nc, in_maps, trace=False, trace_kwargs=None):
    from concourse.bass_interp import get_hw_module

    old = nc.m
    nc.m = get_hw_module(nc.m)
    try:
        res = bass_utils.run_bass_kernel_spmd(
            nc,
            in_maps,
            core_ids=list(range(NCORES)),
            trace=trace,
            **(trace_kwargs or {}),
        )
    finally:
        nc.m = old
    return res


def kernel(queries, keys, seq_len, W_q, W_k, v, b, _trace=False):
    queries = np.asarray(queries, dtype=np.float32)
    keys = np.asarray(keys, dtype=np.float32)
    L_slots, assign, sl = _plan(seq_len)
    nc = _get_prog(L_slots)
    in_maps = _make_in_maps(queries, keys, sl, assign, W_q, W_k, v, b, L_slots)
    res = _run_spmd(nc, in_maps, trace=_trace)
    out = np.zeros((B, TQ, H), np.float32)
    for c in range(NCORES):
        o = res.results[c]["out"]
        for s_i, b_i in enumerate(assign[c]):
            out[b_i] = np.asarray(o[s_i], np.float32)
    # seq_len==0 -> reference softmax degenerates to uniform over all keys
    # (all positions masked to the same NEG_PAD).
    for b_i in np.nonzero(sl == 0)[0]:
        out[b_i] = keys[b_i].mean(axis=0, keepdims=True)
    if _trace:
        kernel._last_results = res
    return out


# revision 44
# speedup vs baseline: 1.2023x; 1.2023x over previous
"""Additive (Bahdanau) attention on Trainium2, data-parallel over batch on 8 NeuronCores.

Math (per batch b):
    qp = queries @ W_q                     [Tq, H]
    kp = keys @ W_k                        [Tk, H]
    scores[q,k] = sum_h v[h] * tanh(qp[q,h] + kp[k,h] + b[h])
    masked softmax over k (k < seq_len[b]), then out = align @ keys.

Design (per core, 4 batch "slots" with compile-time key-lengths L_slots,
ordered [3rd-longest, longest, 2nd-longest, shortest] so the pipeline fills
fast and drains on the smallest slot):
  - ALL host-side layout prep in bf16: transposed keys (ktall), natural
    mask-zeroed keys||maskones chunks (knall), and a const tensor cx holding
    W_q||b rows, [W_k|W_k], v2 and the pair-duplicated transposed queries
    (qTx-even/odd with a trailing ones row) - zero device transposes.
  - qp2+b [128, 128] per slot-pair in TWO K=65 matmuls (even-query half to
    psum[0:64], odd to psum[64:128]; the 65th contraction row is ones x b,
    folding the bias) + one DVE cast to bf16. kp per slot: one matmul
    (lhsT=[W_k|W_k] fills both 64-partition halves) + one DVE cast.
  - S[h2, j*L+k] = kpb2 + qp2[:, 2j+par] via DVE tensor_tensor adds in bf16
    (2x mode), tanh on ACT (the roofline: 32*sum(L) lane-cycles @1.2GHz).
  - scores^T[k, 2j+t] via PE: lhsT = S_tanh[:, j*L+off:+w] stationary,
    rhs = v2 [128, 2] -> psum[w, 64*ci+2j:+2]; all k-chunks of a slot land
    in one PSUM tile, so ONE exp per slot (no bias: masking lives in
    knall's zeroed rows), bf16 out feeding the final matmuls directly.
  - out_un[q, h]|rowsum: PSUM-accumulated E-chunks @ knall chunks; DVE
    reciprocal + scale, bf16 DMA out (host casts back to f32).
  - tanh j-splits are sized so each slot's LAST split is small: the exp
    (which needs every j) then trails the last tanh by only a few score
    matmuls, and the ACT queue interleaves the next slot's tanh to cover
    even that.

Batches are sorted by seq_len and dealt so each core gets one batch per slot
rank; slot k-length = max over the 8 batches of that rank (rounded to 2).
All cores run the identical program on different data (SPMD).
"""

import sys

_REPO = "/opt/trn_rl_repo"
if _REPO not in sys.path:
    sys.path.insert(0, _REPO)

import numpy as np

try:
    from ml_dtypes import bfloat16 as np_bf16
except ImportError:  # pragma: no cover
    import jax.numpy as _jnp

    np_bf16 = _jnp.bfloat16

from concourse import bacc, tile
import concourse.mybir as mybir
from concourse import bass_utils

B, TQ, TK, H = 32, 64, 256, 64
NCORES = 8
SLOTS = 4
F32 = mybir.dt.float32
BF16 = mybir.dt.bfloat16
TANH = mybir.ActivationFunctionType.Tanh
EXP = mybir.ActivationFunctionType.Exp
KXC = 66  # keys | maskones | pad, per 128-row chunk of knall
QPREF = 256  # qTx for slot-pair g=1, prefixed to knall

_prog_cache: dict = {}


def _roundup(x, m):
    return ((x + m - 1) // m) * m


def _nch(L):
    return (L + 127) // 128


# per-slot tanh/adds j-splits: last split small so exp trails it closely
_SLOT_SPLITS = [
    [(0, 8), (8, 20), (20, 32)],
    [(0, 12), (12, 24), (24, 32)],
    [(0, 12), (12, 26), (26, 32)],
    [(0, 16), (16, 26), (26, 32)],
]


def _build(L_slots):
    nc = bacc.Bacc(
        "TRN2",
        target_bir_lowering=False,
        debug=False,
        enable_asserts=False,
        num_devices=NCORES,
    )
    nchs = [_nch(L) for L in L_slots]
    KT = sum(L_slots)
    KN = QPREF + sum(nchs) * KXC
    CX = 194 + 256  # wqb65 64 | wk2 128 | v2 2 | qTx-even/odd g0

    cx_d = nc.dram_tensor("cx", [128, CX], BF16, kind="ExternalInput").ap()
    kt_d = nc.dram_tensor("ktall", [H, KT], BF16, kind="ExternalInput").ap()
    kn_d = nc.dram_tensor("knall", [128, KN], BF16, kind="ExternalInput").ap()
    o_d = nc.dram_tensor("out", [SLOTS, TQ, H], BF16, kind="ExternalOutput").ap()

    ktoff = [sum(L_slots[:s]) for s in range(SLOTS)]
    knoff = [QPREF + sum(nchs[:s]) * KXC for s in range(SLOTS)]
    Lmax = max(L_slots)
    nchmax = max(nchs)

    with tile.TileContext(nc) as tc:
        with (
            tc.tile_pool(name="const", bufs=1) as cpool,
            tc.tile_pool(name="qpool", bufs=2) as qpool,
            tc.tile_pool(name="kpb", bufs=2) as kpool,
            tc.tile_pool(name="spool", bufs=2) as spool,
            tc.tile_pool(name="epool", bufs=2) as epool,
            tc.tile_pool(name="opool", bufs=2) as opool,
            tc.tile_pool(name="qpp", bufs=2, space="PSUM") as qpp,
            tc.tile_pool(name="kpp", bufs=2, space="PSUM") as kpp,
            tc.tile_pool(name="scp", bufs=2, space="PSUM") as scp,
            tc.tile_pool(name="oup", bufs=2, space="PSUM") as oup,
        ):
            # ---- prefetch in three DMAs on three different engines so the
            # three hardware queues stream in parallel (hottest via ACT,
            # whose BSP startup finishes first)
            cx_sb = cpool.tile([128, CX], BF16, name="cx_sb", tag="cx")
            nc.scalar.dma_start(out=cx_sb, in_=cx_d)
            kt_sb = cpool.tile([H, KT], BF16, name="kt_sb", tag="kt")
            nc.sync.dma_start(out=kt_sb, in_=kt_d)
            kn_sb = cpool.tile([128, KN], BF16, name="kn_sb", tag="kn")
            nc.gpsimd.dma_start(out=kn_sb, in_=kn_d)

            # tiny activation so the ACT table set loads during the DMA wait
            scr = cpool.tile([1, 2], F32, name="scr", tag="scr")
            nc.vector.memset(scr, 0.0)
            nc.scalar.activation(scr, scr, TANH)

            wqb65 = cx_sb[0:65, 0:64]
            wk2_sb = cx_sb[0:H, 64:192]
            v2_sb = cx_sb[:, 192:194]
            qtx = {
                (0, 0): cx_sb[0:65, 194:322],
                (0, 1): cx_sb[0:65, 322:450],
                (1, 0): kn_sb[0:65, 0:128],
                (1, 1): kn_sb[0:65, 128:256],
            }

            qp2_sb = [None, None]

            def qproj_mm(g):
                ps = qpp.tile([128, 128], F32, name=f"qpps{g}", tag="qp")
                nc.tensor.matmul(ps[0:64, :], lhsT=wqb65, rhs=qtx[(g, 0)])
                nc.tensor.matmul(ps[64:128, :], lhsT=wqb65, rhs=qtx[(g, 1)])
                return ps

            def qproj_dve(g, ps):
                qp2 = qpool.tile([128, 128], BF16, name=f"qp2_{g}", tag="qp2")
                nc.vector.tensor_copy(qp2, ps)
                qp2_sb[g] = qp2

            kpb2s = {}

            def kproj_mm(s):
                L = L_slots[s]
                ps = kpp.tile([128, Lmax], F32, name=f"kpps{s}", tag="kp")
                nc.tensor.matmul(
                    ps[:, 0:L], lhsT=wk2_sb, rhs=kt_sb[:, ktoff[s] : ktoff[s] + L]
                )
                return ps

            def kproj_dve(s, ps):
                L = L_slots[s]
                kpb2 = kpool.tile([128, Lmax], BF16, name=f"kpb2_{s}", tag="kpb2")
                nc.vector.tensor_copy(kpb2[:, 0:L], ps[:, 0:L])
                kpb2s[s] = kpb2

            # ---- per-slot stages
            S_alls, S_tanhs, sc_pss, E_sbs, out_pss = {}, {}, {}, {}, {}

            def adds(s, j0, j1):
                L = L_slots[s]
                if s not in S_alls:
                    S_alls[s] = spool.tile(
                        [128, 32 * Lmax], BF16, name=f"S{s}", tag="S"
                    )
                S_all = S_alls[s]
                kpb2 = kpb2s[s]
                qp2 = qp2_sb[s // 2]
                qoff = 64 * (s % 2)
                nj = j1 - j0
                in0 = (
                    kpb2[:, 0:L]
                    .rearrange("c (k two) -> c k two", two=2)
                    .unsqueeze(1)
                    .broadcast_to([128, nj, L // 2, 2])
                )
                in1 = (
                    qp2[:, qoff + 2 * j0 : qoff + 2 * j1]
                    .rearrange("c (j two) -> c j two", two=2)
                    .unsqueeze(2)
                    .broadcast_to([128, nj, L // 2, 2])
                )
                s_out = S_all[:, j0 * L : j1 * L].rearrange(
                    "c (j k two) -> c j k two", two=2, k=L // 2
                )
                nc.vector.tensor_add(s_out, in0, in1)

            def tanh(s, j0, j1):
                L = L_slots[s]
                if s not in S_tanhs:
                    S_tanhs[s] = spool.tile(
                        [128, 32 * Lmax], BF16, name=f"T{s}", tag="T"
                    )
                nc.scalar.activation(
                    S_tanhs[s][:, j0 * L : j1 * L],
                    S_alls[s][:, j0 * L : j1 * L],
                    TANH,
                )

            def scores(s, j0, j1):
                L = L_slots[s]
                nch = nchs[s]
                S_tanh = S_tanhs[s]
                if s not in sc_pss:
                    sc_pss[s] = scp.tile(
                        [128, 64 * nchmax], F32, name=f"sc{s}", tag="sc"
                    )
                    wl = L - 128 * (nch - 1)
                    if nch > 1 and wl < 128:
                        # exp reads the whole tile; pre-zero the last chunk's
                        # columns (its matmuls overwrite rows [0:wl])
                        nc.vector.memset(
                            sc_pss[s][:, 64 * (nch - 1) : 64 * nch], 0.0
                        )
                sc_ps = sc_pss[s]
                for ci in range(nch):
                    off = 128 * ci
                    w = min(128, L - off)
                    for j in range(j0, j1):
                        nc.tensor.matmul(
                            sc_ps[0:w, 64 * ci + 2 * j : 64 * ci + 2 * j + 2],
                            lhsT=S_tanh[:, j * L + off : j * L + off + w],
                            rhs=v2_sb,
                            start=True,
                            stop=True,
                        )

            def expo(s, j0=0, j1=32):
                # exp of query-columns [2*j0, 2*j1) across all k-chunks; a
                # partial call lets the last slot's exp trail its last tanh
                # by only a few score matmuls
                L = L_slots[s]
                nch = nchs[s]
                wmax = min(128, L)
                if s not in E_sbs:
                    E_sbs[s] = epool.tile(
                        [128, 64 * nchmax], BF16, name=f"E{s}", tag="E"
                    )
                E = E_sbs[s]
                if (j0, j1) == (0, 32):
                    nc.scalar.activation(
                        E[0:wmax, 0 : 64 * nch], sc_pss[s][0:wmax, 0 : 64 * nch], EXP
                    )
                    return
                for ci in range(nch):
                    c0, c1 = 64 * ci + 2 * j0, 64 * ci + 2 * j1
                    nc.scalar.activation(
                        E[0:wmax, c0:c1], sc_pss[s][0:wmax, c0:c1], EXP
                    )

            def outmm(s):
                L = L_slots[s]
                nch = nchs[s]
                E = E_sbs[s]
                out_ps = oup.tile([TQ, H + 1], F32, name=f"ops{s}", tag="ou")
                out_pss[s] = out_ps
                for ci in range(nch):
                    w = min(128, L - 128 * ci)
                    kno = knoff[s] + ci * KXC
                    nc.tensor.matmul(
                        out_ps,
                        lhsT=E[0:w, 64 * ci : 64 * ci + 64],
                        rhs=kn_sb[0:w, kno : kno + H + 1],
                        start=(ci == 0),
                        stop=(ci == nch - 1),
                    )

            def norm(s):
                out_ps = out_pss[s]
                recip = opool.tile([TQ, 1], F32, name=f"rc{s}", tag="rc")
                nc.vector.reciprocal(recip, out_ps[:, H : H + 1])
                out_sb = opool.tile([TQ, H], BF16, name=f"osb{s}", tag="osb")
                nc.vector.tensor_scalar_mul(out_sb, out_ps[:, 0:H], recip)
                nc.sync.dma_start(out=o_d[s], in_=out_sb)

            sp = _SLOT_SPLITS

            # ---- hand-interleaved schedule.
            # ACT: W,T0abc,T1a,E0,T1b,T1c,T2a,E1,T2b,T3a,T2c,T3b,E2,T3c,E3
            # DVE: casts just-in-time, adds in ACT order, norms at the end.
            # PE:  projections early, scores in j-split order, outmms asap.
            kps0 = kproj_mm(0)
            kproj_dve(0, kps0)
            qps0 = qproj_mm(0)
            qproj_dve(0, qps0)
            kps1 = kproj_mm(1)

            for j0, j1 in sp[0]:
                adds(0, j0, j1)
                tanh(0, j0, j1)
                scores(0, j0, j1)
            kproj_dve(1, kps1)
            qps1 = qproj_mm(1)
            kps2 = kproj_mm(2)

            adds(1, *sp[1][0])
            tanh(1, *sp[1][0])
            expo(0)
            scores(1, *sp[1][0])
            adds(1, *sp[1][1])
            tanh(1, *sp[1][1])
            scores(1, *sp[1][1])
            qproj_dve(1, qps1)
            adds(1, *sp[1][2])
            tanh(1, *sp[1][2])
            scores(1, *sp[1][2])
            kproj_dve(2, kps2)
            kps3 = kproj_mm(3)
            outmm(0)

            adds(2, *sp[2][0])
            tanh(2, *sp[2][0])
            expo(1)
            scores(2, *sp[2][0])
            adds(2, *sp[2][1])
            tanh(2, *sp[2][1])
            scores(2, *sp[2][1])
            kproj_dve(3, kps3)
            outmm(1)

            adds(3, *sp[3][0])
            tanh(3, *sp[3][0])
            adds(2, *sp[2][2])
            tanh(2, *sp[2][2])
            scores(2, *sp[2][2])
            adds(3, *sp[3][1])
            tanh(3, *sp[3][1])
            scores(3, *sp[3][0])
            scores(3, *sp[3][1])
            expo(2)
            adds(3, *sp[3][2])
            tanh(3, *sp[3][2])
            scores(3, *sp[3][2])
            outmm(2)
            expo(3)
            outmm(3)
            for s in range(SLOTS):
                norm(s)

    nc.compile()
    return nc


def _get_prog(L_slots):
    if L_slots not in _prog_cache:
        _prog_cache[L_slots] = _build(L_slots)
    return _prog_cache[L_slots]


def _plan(seq_len_flat):
    sl = np.asarray(seq_len_flat).reshape(-1).astype(np.int64)
    order = np.argsort(-sl, kind="stable")
    grp = [order[NCORES * r : NCORES * (r + 1)] for r in range(SLOTS)]
    slot_of_rank = [1, 2, 0, 3]  # rank r (0=longest) -> slot index
    assign = np.zeros((NCORES, SLOTS), dtype=np.int64)
    L_slots = [0] * SLOTS
    for r in range(SLOTS):
        s = slot_of_rank[r]
        assign[:, s] = grp[r]
        L = int(max(1, sl[grp[r]].max()))
        L_slots[s] = min(TK, _roundup(L, 2))
    return tuple(L_slots), assign, sl


def _make_in_maps(queries, keys, sl, assign, W_q, W_k, v, b, L_slots):
    W_q = np.asarray(W_q, np.float32)
    W_k = np.asarray(W_k, np.float32)
    vv = np.asarray(v, np.float32).reshape(-1)
    bb = np.asarray(b, np.float32).reshape(-1)
    nchs = [_nch(L) for L in L_slots]
    KT = sum(L_slots)
    KN = QPREF + sum(nchs) * KXC
    CX = 194 + 256

    base = np.zeros((128, CX), np.float32)
    base[0:H, 0:H] = W_q
    base[H, 0:H] = bb
    base[0:H, 64 : 64 + H] = W_k
    base[0:H, 64 + H : 192] = W_k
    base[0:H, 192] = vv
    base[H:128, 193] = vv

    def qtx_cols(q):
        # [65, 64]: row h = queries[2*(m//2) (+1 for odd), h].T; row 64 = 1
        up = np.zeros((65, 64), np.float32)
        lo = np.zeros((65, 64), np.float32)
        up[0:H] = np.repeat(q[0::2], 2, axis=0).T
        lo[0:H] = np.repeat(q[1::2], 2, axis=0).T
        up[H] = 1.0
        lo[H] = 1.0
        return up, lo

    in_maps = []
    for c in range(NCORES):
        cx = base.copy()
        knall = np.zeros((128, KN), np.float32)
        for g in range(2):
            for half in range(2):
                s = 2 * g + half
                up, lo = qtx_cols(queries[assign[c, s]])
                if g == 0:
                    cx[0:65, 194 + 64 * half : 258 + 64 * half] = up
                    cx[0:65, 322 + 64 * half : 386 + 64 * half] = lo
                else:
                    knall[0:65, 64 * half : 64 * half + 64] = up
                    knall[0:65, 128 + 64 * half : 192 + 64 * half] = lo
        ktall = np.zeros((H, KT), np.float32)
        kto, kno = 0, QPREF
        for s, L in enumerate(L_slots):
            b_i = assign[c, s]
            kk = keys[b_i]  # [256, 64]
            lv = int(min(sl[b_i], L))
            ktall[:, kto : kto + lv] = kk[0:lv].T
            for ci in range(nchs[s]):
                off = 128 * ci
                w = min(128, L - off)
                vw = max(0, min(lv - off, w))
                if vw > 0:
                    knall[0:vw, kno : kno + H] = kk[off : off + vw]
                    knall[0:vw, kno + H] = 1.0
                kno += KXC
            kto += L
        in_maps.append(
            {
                "cx": cx.astype(np_bf16),
                "ktall": ktall.astype(np_bf16),
                "knall": knall.astype(np_bf16),
            }
        )
    return in_maps


def _run_spmd(nc, in_maps, trace=False, trace_kwargs=None):
    from concourse.bass_interp import get_hw_module

    old = nc.m
    nc.m = get_hw_module(nc.m)
    try:
        res = bass_utils.run_bass_kernel_spmd(
            nc,
            in_maps,
            core_ids=list(range(NCORES)),
            trace=trace,
            **(trace_kwargs or {}),
        )
    finally:
        nc.m = old
    return res


def kernel(queries, keys, seq_len, W_q, W_k, v, b, _trace=False):
    queries = np.asarray(queries, dtype=np.float32)
    keys = np.asarray(keys, dtype=np.float32)
    L_slots, assign, sl = _plan(seq_len)
    nc = _get_prog(L_slots)
    in_maps = _make_in_maps(queries, keys, sl, assign, W_q, W_k, v, b, L_slots)
    res = _run_spmd(nc, in_maps, trace=_trace)
    out = np.zeros((B, TQ, H), np.float32)
    for c in range(NCORES):
        o = res.results[c]["out"]
        for s_i, b_i in enumerate(assign[c]):
            out[b_i] = np.asarray(o[s_i], np.float32)
    # seq_len==0 -> reference softmax degenerates to uniform over all keys
    # (all positions masked to the same NEG_PAD).
    for b_i in np.nonzero(sl == 0)[0]:
        out[b_i] = keys[b_i].mean(axis=0, keepdims=True)
    if _trace:
        kernel._last_results = res
    return out


# revision 45
# speedup vs baseline: 1.2094x; 1.0059x over previous
"""Additive (Bahdanau) attention on Trainium2, data-parallel over batch on 8 NeuronCores.

Math (per batch b):
    qp = queries @ W_q                     [Tq, H]
    kp = keys @ W_k                        [Tk, H]
    scores[q,k] = sum_h v[h] * tanh(qp[q,h] + kp[k,h] + b[h])
    masked softmax over k (k < seq_len[b]), then out = align @ keys.

Design (per core, 4 batch "slots" with compile-time key-lengths L_slots,
ordered [3rd-longest, longest, 2nd-longest, shortest] so the pipeline fills
fast and drains on the smallest slot):
  - ALL host-side layout prep in bf16: transposed keys (ktall), natural
    mask-zeroed keys||maskones chunks (knall), and a const tensor cx holding
    W_q||b rows, [W_k|W_k], v2 and the pair-duplicated transposed queries
    (qTx-even/odd with a trailing ones row) - zero device transposes.
  - qp2+b [128, 128] per slot-pair in TWO K=65 matmuls (even-query half to
    psum[0:64], odd to psum[64:128]; the 65th contraction row is ones x b,
    folding the bias) + one DVE cast to bf16. kp per slot: one matmul
    (lhsT=[W_k|W_k] fills both 64-partition halves) + one DVE cast.
  - S[h2, j*L+k] = kpb2 + qp2[:, 2j+par] via DVE tensor_tensor adds in bf16
    (2x mode), tanh on ACT (the roofline: 32*sum(L) lane-cycles @1.2GHz).
  - scores^T[k, 2j+t] via PE: lhsT = S_tanh[:, j*L+off:+w] stationary,
    rhs = v2 [128, 2] -> psum[w, 64*ci+2j:+2]; all k-chunks of a slot land
    in one PSUM tile, so ONE exp per slot (no bias: masking lives in
    knall's zeroed rows), bf16 out feeding the final matmuls directly.
  - out_un[q, h]|rowsum: PSUM-accumulated E-chunks @ knall chunks; DVE
    reciprocal + scale, bf16 DMA out (host casts back to f32).
  - tanh j-splits are sized so each slot's LAST split is small: the exp
    (which needs every j) then trails the last tanh by only a few score
    matmuls, and the ACT queue interleaves the next slot's tanh to cover
    even that.

Batches are sorted by seq_len and dealt so each core gets one batch per slot
rank; slot k-length = max over the 8 batches of that rank (rounded to 2).
All cores run the identical program on different data (SPMD).
"""

import sys

_REPO = "/opt/trn_rl_repo"
if _REPO not in sys.path:
    sys.path.insert(0, _REPO)

import numpy as np

try:
    from ml_dtypes import bfloat16 as np_bf16
except ImportError:  # pragma: no cover
    import jax.numpy as _jnp

    np_bf16 = _jnp.bfloat16

from concourse import bacc, tile
import concourse.mybir as mybir
from concourse import bass_utils

B, TQ, TK, H = 32, 64, 256, 64
NCORES = 8
SLOTS = 4
F32 = mybir.dt.float32
BF16 = mybir.dt.bfloat16
TANH = mybir.ActivationFunctionType.Tanh
EXP = mybir.ActivationFunctionType.Exp
KXC = 66  # keys | maskones | pad, per 128-row chunk of knall
QPREF = 256  # qTx for slot-pair g=1, prefixed to knall

_prog_cache: dict = {}


def _roundup(x, m):
    return ((x + m - 1) // m) * m


def _nch(L):
    return (L + 127) // 128


# per-slot tanh/adds j-splits: last split small so exp trails it closely
_SLOT_SPLITS = [
    [(0, 6), (6, 18), (18, 32)],
    [(0, 12), (12, 24), (24, 32)],
    [(0, 12), (12, 26), (26, 32)],
    [(0, 16), (16, 26), (26, 32)],
]


def _build(L_slots):
    nc = bacc.Bacc(
        "TRN2",
        target_bir_lowering=False,
        debug=False,
        enable_asserts=False,
        num_devices=NCORES,
    )
    nchs = [_nch(L) for L in L_slots]
    KT = sum(L_slots)
    KN = QPREF + sum(nchs) * KXC
    CX = 194 + 256  # wqb65 64 | wk2 128 | v2 2 | qTx-even/odd g0

    cx_d = nc.dram_tensor("cx", [128, CX], BF16, kind="ExternalInput").ap()
    kt_d = nc.dram_tensor("ktall", [H, KT], BF16, kind="ExternalInput").ap()
    kn_d = nc.dram_tensor("knall", [128, KN], BF16, kind="ExternalInput").ap()
    o_d = nc.dram_tensor("out", [SLOTS, TQ, H], BF16, kind="ExternalOutput").ap()

    ktoff = [sum(L_slots[:s]) for s in range(SLOTS)]
    knoff = [QPREF + sum(nchs[:s]) * KXC for s in range(SLOTS)]
    Lmax = max(L_slots)
    nchmax = max(nchs)

    with tile.TileContext(nc) as tc:
        with (
            tc.tile_pool(name="const", bufs=1) as cpool,
            tc.tile_pool(name="qpool", bufs=2) as qpool,
            tc.tile_pool(name="kpb", bufs=2) as kpool,
            tc.tile_pool(name="spool", bufs=2) as spool,
            tc.tile_pool(name="epool", bufs=2) as epool,
            tc.tile_pool(name="opool", bufs=2) as opool,
            tc.tile_pool(name="qpp", bufs=2, space="PSUM") as qpp,
            tc.tile_pool(name="kpp", bufs=2, space="PSUM") as kpp,
            tc.tile_pool(name="scp", bufs=2, space="PSUM") as scp,
            tc.tile_pool(name="oup", bufs=2, space="PSUM") as oup,
        ):
            # ---- prefetch in three DMAs on three different engines so the
            # three hardware queues stream in parallel (hottest via ACT,
            # whose BSP startup finishes first)
            cx_sb = cpool.tile([128, CX], BF16, name="cx_sb", tag="cx")
            nc.scalar.dma_start(out=cx_sb, in_=cx_d)
            kt_sb = cpool.tile([H, KT], BF16, name="kt_sb", tag="kt")
            nc.sync.dma_start(out=kt_sb, in_=kt_d)
            kn_sb = cpool.tile([128, KN], BF16, name="kn_sb", tag="kn")
            nc.gpsimd.dma_start(out=kn_sb, in_=kn_d)

            # tiny activation so the ACT table set loads during the DMA wait
            scr = cpool.tile([1, 2], F32, name="scr", tag="scr")
            nc.vector.memset(scr, 0.0)
            nc.scalar.activation(scr, scr, TANH)

            wqb65 = cx_sb[0:65, 0:64]
            wk2_sb = cx_sb[0:H, 64:192]
            v2_sb = cx_sb[:, 192:194]
            qtx = {
                (0, 0): cx_sb[0:65, 194:322],
                (0, 1): cx_sb[0:65, 322:450],
                (1, 0): kn_sb[0:65, 0:128],
                (1, 1): kn_sb[0:65, 128:256],
            }

            qp2_sb = [None, None]

            def qproj_mm(g):
                ps = qpp.tile([128, 128], F32, name=f"qpps{g}", tag="qp")
                nc.tensor.matmul(ps[0:64, :], lhsT=wqb65, rhs=qtx[(g, 0)])
                nc.tensor.matmul(ps[64:128, :], lhsT=wqb65, rhs=qtx[(g, 1)])
                return ps

            def qproj_dve(g, ps):
                qp2 = qpool.tile([128, 128], BF16, name=f"qp2_{g}", tag="qp2")
                nc.vector.tensor_copy(qp2, ps)
                qp2_sb[g] = qp2

            kpb2s = {}

            def kproj_mm(s):
                L = L_slots[s]
                ps = kpp.tile([128, Lmax], F32, name=f"kpps{s}", tag="kp")
                nc.tensor.matmul(
                    ps[:, 0:L], lhsT=wk2_sb, rhs=kt_sb[:, ktoff[s] : ktoff[s] + L]
                )
                return ps

            def kproj_dve(s, ps):
                L = L_slots[s]
                kpb2 = kpool.tile([128, Lmax], BF16, name=f"kpb2_{s}", tag="kpb2")
                nc.vector.tensor_copy(kpb2[:, 0:L], ps[:, 0:L])
                kpb2s[s] = kpb2

            # ---- per-slot stages
            S_alls, S_tanhs, sc_pss, E_sbs, out_pss = {}, {}, {}, {}, {}

            def adds(s, j0, j1):
                L = L_slots[s]
                if s not in S_alls:
                    S_alls[s] = spool.tile(
                        [128, 32 * Lmax], BF16, name=f"S{s}", tag="S"
                    )
                S_all = S_alls[s]
                kpb2 = kpb2s[s]
                qp2 = qp2_sb[s // 2]
                qoff = 64 * (s % 2)
                nj = j1 - j0
                in0 = (
                    kpb2[:, 0:L]
                    .rearrange("c (k two) -> c k two", two=2)
                    .unsqueeze(1)
                    .broadcast_to([128, nj, L // 2, 2])
                )
                in1 = (
                    qp2[:, qoff + 2 * j0 : qoff + 2 * j1]
                    .rearrange("c (j two) -> c j two", two=2)
                    .unsqueeze(2)
                    .broadcast_to([128, nj, L // 2, 2])
                )
                s_out = S_all[:, j0 * L : j1 * L].rearrange(
                    "c (j k two) -> c j k two", two=2, k=L // 2
                )
                nc.vector.tensor_add(s_out, in0, in1)

            def tanh(s, j0, j1):
                L = L_slots[s]
                if s not in S_tanhs:
                    S_tanhs[s] = spool.tile(
                        [128, 32 * Lmax], BF16, name=f"T{s}", tag="T"
                    )
                nc.scalar.activation(
                    S_tanhs[s][:, j0 * L : j1 * L],
                    S_alls[s][:, j0 * L : j1 * L],
                    TANH,
                )

            def scores(s, j0, j1):
                L = L_slots[s]
                nch = nchs[s]
                S_tanh = S_tanhs[s]
                if s not in sc_pss:
                    sc_pss[s] = scp.tile(
                        [128, 64 * nchmax], F32, name=f"sc{s}", tag="sc"
                    )
                    wl = L - 128 * (nch - 1)
                    if nch > 1 and wl < 128:
                        # exp reads the whole tile; pre-zero the last chunk's
                        # columns (its matmuls overwrite rows [0:wl])
                        nc.vector.memset(
                            sc_pss[s][:, 64 * (nch - 1) : 64 * nch], 0.0
                        )
                sc_ps = sc_pss[s]
                for ci in range(nch):
                    off = 128 * ci
                    w = min(128, L - off)
                    for j in range(j0, j1):
                        nc.tensor.matmul(
                            sc_ps[0:w, 64 * ci + 2 * j : 64 * ci + 2 * j + 2],
                            lhsT=S_tanh[:, j * L + off : j * L + off + w],
                            rhs=v2_sb,
                            start=True,
                            stop=True,
                        )

            def expo(s, j0=0, j1=32):
                # exp of query-columns [2*j0, 2*j1) across all k-chunks; a
                # partial call lets the last slot's exp trail its last tanh
                # by only a few score matmuls
                L = L_slots[s]
                nch = nchs[s]
                wmax = min(128, L)
                if s not in E_sbs:
                    E_sbs[s] = epool.tile(
                        [128, 64 * nchmax], BF16, name=f"E{s}", tag="E"
                    )
                E = E_sbs[s]
                if (j0, j1) == (0, 32):
                    nc.scalar.activation(
                        E[0:wmax, 0 : 64 * nch], sc_pss[s][0:wmax, 0 : 64 * nch], EXP
                    )
                    return
                for ci in range(nch):
                    c0, c1 = 64 * ci + 2 * j0, 64 * ci + 2 * j1
                    nc.scalar.activation(
                        E[0:wmax, c0:c1], sc_pss[s][0:wmax, c0:c1], EXP
                    )

            def outmm(s):
                L = L_slots[s]
                nch = nchs[s]
                E = E_sbs[s]
                out_ps = oup.tile([TQ, H + 1], F32, name=f"ops{s}", tag="ou")
                out_pss[s] = out_ps
                for ci in range(nch):
                    w = min(128, L - 128 * ci)
                    kno = knoff[s] + ci * KXC
                    nc.tensor.matmul(
                        out_ps,
                        lhsT=E[0:w, 64 * ci : 64 * ci + 64],
                        rhs=kn_sb[0:w, kno : kno + H + 1],
                        start=(ci == 0),
                        stop=(ci == nch - 1),
                    )

            def norm(s):
                out_ps = out_pss[s]
                recip = opool.tile([TQ, 1], F32, name=f"rc{s}", tag="rc")
                nc.vector.reciprocal(recip, out_ps[:, H : H + 1])
                out_sb = opool.tile([TQ, H], BF16, name=f"osb{s}", tag="osb")
                nc.vector.tensor_scalar_mul(out_sb, out_ps[:, 0:H], recip)
                nc.sync.dma_start(out=o_d[s], in_=out_sb)

            sp = _SLOT_SPLITS

            # ---- hand-interleaved schedule.
            # ACT: W,T0abc,T1a,E0,T1b,T1c,T2a,E1,T2b,T3a,T2c,T3b,E2,T3c,E3
            # DVE: casts just-in-time, adds in ACT order, norms at the end.
            # PE:  projections early, scores in j-split order, outmms asap.
            kps0 = kproj_mm(0)
            kproj_dve(0, kps0)
            qps0 = qproj_mm(0)
            qproj_dve(0, qps0)
            kps1 = kproj_mm(1)

            for j0, j1 in sp[0]:
                adds(0, j0, j1)
                tanh(0, j0, j1)
                scores(0, j0, j1)
            kproj_dve(1, kps1)
            qps1 = qproj_mm(1)
            kps2 = kproj_mm(2)

            adds(1, *sp[1][0])
            tanh(1, *sp[1][0])
            expo(0)
            scores(1, *sp[1][0])
            adds(1, *sp[1][1])
            tanh(1, *sp[1][1])
            scores(1, *sp[1][1])
            qproj_dve(1, qps1)
            adds(1, *sp[1][2])
            tanh(1, *sp[1][2])
            scores(1, *sp[1][2])
            kproj_dve(2, kps2)
            kps3 = kproj_mm(3)
            outmm(0)

            adds(2, *sp[2][0])
            tanh(2, *sp[2][0])
            expo(1)
            scores(2, *sp[2][0])
            adds(2, *sp[2][1])
            tanh(2, *sp[2][1])
            scores(2, *sp[2][1])
            kproj_dve(3, kps3)
            outmm(1)

            adds(3, *sp[3][0])
            tanh(3, *sp[3][0])
            adds(2, *sp[2][2])
            tanh(2, *sp[2][2])
            scores(2, *sp[2][2])
            adds(3, *sp[3][1])
            tanh(3, *sp[3][1])
            scores(3, *sp[3][0])
            scores(3, *sp[3][1])
            expo(2)
            adds(3, *sp[3][2])
            tanh(3, *sp[3][2])
            scores(3, *sp[3][2])
            outmm(2)
            expo(3)
            outmm(3)
            for s in range(SLOTS):
                norm(s)

    nc.compile()
    return nc


def _get_prog(L_slots):
    if L_slots not in _prog_cache:
        _prog_cache[L_slots] = _build(L_slots)
    return _prog_cache[L_slots]


def _plan(seq_len_flat):
    sl = np.asarray(seq_len_flat).reshape(-1).astype(np.int64)
    order = np.argsort(-sl, kind="stable")
    grp = [order[NCORES * r : NCORES * (r + 1)] for r in range(SLOTS)]
    slot_of_rank = [1, 2, 0, 3]  # rank r (0=longest) -> slot index
    assign = np.zeros((NCORES, SLOTS), dtype=np.int64)
    L_slots = [0] * SLOTS
    for r in range(SLOTS):
        s = slot_of_rank[r]
        assign[:, s] = grp[r]
        L = int(max(1, sl[grp[r]].max()))
        L_slots[s] = min(TK, _roundup(L, 2))
    return tuple(L_slots), assign, sl


def _make_in_maps(queries, keys, sl, assign, W_q, W_k, v, b, L_slots):
    W_q = np.asarray(W_q, np.float32)
    W_k = np.asarray(W_k, np.float32)
    vv = np.asarray(v, np.float32).reshape(-1)
    bb = np.asarray(b, np.float32).reshape(-1)
    nchs = [_nch(L) for L in L_slots]
    KT = sum(L_slots)
    KN = QPREF + sum(nchs) * KXC
    CX = 194 + 256

    base = np.zeros((128, CX), np.float32)
    base[0:H, 0:H] = W_q
    base[H, 0:H] = bb
    base[0:H, 64 : 64 + H] = W_k
    base[0:H, 64 + H : 192] = W_k
    base[0:H, 192] = vv
    base[H:128, 193] = vv

    def qtx_cols(q):
        # [65, 64]: row h = queries[2*(m//2) (+1 for odd), h].T; row 64 = 1
        up = np.zeros((65, 64), np.float32)
        lo = np.zeros((65, 64), np.float32)
        up[0:H] = np.repeat(q[0::2], 2, axis=0).T
        lo[0:H] = np.repeat(q[1::2], 2, axis=0).T
        up[H] = 1.0
        lo[H] = 1.0
        return up, lo

    in_maps = []
    for c in range(NCORES):
        cx = base.copy()
        knall = np.zeros((128, KN), np.float32)
        for g in range(2):
            for half in range(2):
                s = 2 * g + half
                up, lo = qtx_cols(queries[assign[c, s]])
                if g == 0:
                    cx[0:65, 194 + 64 * half : 258 + 64 * half] = up
                    cx[0:65, 322 + 64 * half : 386 + 64 * half] = lo
                else:
                    knall[0:65, 64 * half : 64 * half + 64] = up
                    knall[0:65, 128 + 64 * half : 192 + 64 * half] = lo
        ktall = np.zeros((H, KT), np.float32)
        kto, kno = 0, QPREF
        for s, L in enumerate(L_slots):
            b_i = assign[c, s]
            kk = keys[b_i]  # [256, 64]
            lv = int(min(sl[b_i], L))
            ktall[:, kto : kto + lv] = kk[0:lv].T
            for ci in range(nchs[s]):
                off = 128 * ci
                w = min(128, L - off)
                vw = max(0, min(lv - off, w))
                if vw > 0:
                    knall[0:vw, kno : kno + H] = kk[off : off + vw]
                    knall[0:vw, kno + H] = 1.0
                kno += KXC
            kto += L
        in_maps.append(
            {
                "cx": cx.astype(np_bf16),
                "ktall": ktall.astype(np_bf16),
                "knall": knall.astype(np_bf16),
            }
        )
    return in_maps


def _run_spmd(nc, in_maps, trace=False, trace_kwargs=None):
    from concourse.bass_interp import get_hw_module

    old = nc.m
    nc.m = get_hw_module(nc.m)
    try:
        res = bass_utils.run_bass_kernel_spmd(
            nc,
            in_maps,
            core_ids=list(range(NCORES)),
            trace=trace,
            **(trace_kwargs or {}),
        )
    finally:
        nc.m = old
    return res


def kernel(queries, keys, seq_len, W_q, W_k, v, b, _trace=False):
    queries = np.asarray(queries, dtype=np.float32)
    keys = np.asarray(keys, dtype=np.float32)
    L_slots, assign, sl = _plan(seq_len)
    nc = _get_prog(L_slots)
    in_maps = _make_in_maps(queries, keys, sl, assign, W_q, W_k, v, b, L_slots)
    res = _run_spmd(nc, in_maps, trace=_trace)
    out = np.zeros((B, TQ, H), np.float32)
    for c in range(NCORES):
        o = res.results[c]["out"]
        for s_i, b_i in enumerate(assign[c]):
            out[b_i] = np.asarray(o[s_i], np.float32)
    # seq_len==0 -> reference softmax degenerates to uniform over all keys
    # (all positions masked to the same NEG_PAD).
    for b_i in np.nonzero(sl == 0)[0]:
        out[b_i] = keys[b_i].mean(axis=0, keepdims=True)
    if _trace:
        kernel._last_results = res
    return out
